# revision 8
# baseline (speedup 1.0000x reference)
"""Mixtral sparse MoE block on 8 Trainium2 NeuronCores (expert-parallel).

Strategy
--------
Each of the 8 cores owns one expert e (= its position in the SPMD in_maps
list).  Every core:
  1. computes the router for ALL 2048 tokens in fp32 on device
     (PE-transpose of x, x^T @ gate_w^T, top-2 via the DVE max8 op),
  2. compacts the tokens routed to its expert with a matmul-based cumsum
     and an indirect-DMA scatter of "token records" (x row | weight |
     token id) into a per-core DRAM buffer with capacity C=640,
  3. runs the SwiGLU expert MLP on the gathered tokens in bf16
     (fp32 PSUM accumulation),
  4. scatters weight-scaled outputs back to the (pre-zeroed) output
     tensor by original token index.
The host sums the 8 partial outputs (pure unshard/reduce of the
expert-parallel sharding).

kernel(**inputs) takes the FULL unsharded inputs and returns the FULL
output, as required.
"""

import sys

for _p in ("/opt/trn_rl_repo",):
    if _p not in sys.path:
        sys.path.insert(0, _p)

import numpy as np
import ml_dtypes

import concourse.bass as bass
import concourse.mybir as mybir
import concourse.tile as tile
from concourse import bacc
from concourse.bass import IndirectOffsetOnAxis
from concourse.bass_utils import run_bass_kernel_spmd
from concourse.masks import make_identity

AF = mybir.ActivationFunctionType
ALU = mybir.AluOpType
F32 = mybir.dt.float32
BF16 = mybir.dt.bfloat16
I32 = mybir.dt.int32

BF16_NP = ml_dtypes.bfloat16

# Problem geometry (hardcoded per contract)
T = 2048          # tokens (batch 1 x seq 2048)
H = 2048          # hidden
I = 7168          # expert ffn dim
E = 8             # experts (= cores)
P = 128           # partitions
NT = T // P       # 16 token tiles
NH = H // P       # 16 hidden tiles
NI = I // P       # 56 ffn tiles
C = 640           # per-expert token capacity (mean load 512, sigma ~21)
NC_TILES = C // P  # 5
IB = 512          # stage-1 ffn block (columns of w1t/w3t per load)
NIB = I // IB     # 14
JB = 4            # stage-2 ffn tiles per load (4 x 128 = 512 rows of w2t)
NJB = NI // JB    # 14
REC = 2056        # token record width: 2048 x | 1 we | 1 tid | 6 pad
BIG = 65536.0     # scatter position for unselected tokens (dropped via bounds)

N_CORES = 8


def _build():
    nc = bacc.Bacc()
    x_d = nc.dram_tensor("x", [T, H], F32, kind="ExternalInput")
    gwt_d = nc.dram_tensor("gwt", [P, NH, E], F32, kind="ExternalInput")
    eh_d = nc.dram_tensor("eh", [P, E], F32, kind="ExternalInput")
    tri_d = nc.dram_tensor("tri", [P, P], F32, kind="ExternalInput")
    w1p_d = nc.dram_tensor("w1p", [NIB, NH, P, IB], BF16, kind="ExternalInput")
    w3p_d = nc.dram_tensor("w3p", [NIB, NH, P, IB], BF16, kind="ExternalInput")
    w2t_d = nc.dram_tensor("w2t", [I, H], BF16, kind="ExternalInput")
    out_d = nc.dram_tensor("out", [T, H], F32, kind="ExternalOutput")

    with tile.TileContext(nc) as tc:
        with (
            tc.tile_pool(name="const", bufs=1) as cp,
            tc.tile_pool(name="dram", bufs=1, space="DRAM") as dp,
        ):
            ident = cp.tile([P, P], F32)
            make_identity(nc, ident[:])
            ident_bf = cp.tile([P, P], BF16)
            make_identity(nc, ident_bf[:])
            ones = cp.tile([P, P], F32)
            nc.vector.memset(ones[:], 1.0)
            tri_sb = cp.tile([P, P], F32)
            nc.sync.dma_start(tri_sb[:], tri_d[:])
            gwt_sb = cp.tile([P, NH, E], F32)
            nc.sync.dma_start(gwt_sb[:], gwt_d[:])
            eh_sb = cp.tile([P, E], F32)
            nc.sync.dma_start(eh_sb[:], eh_d[:])

            # Phase A results that must survive into later phases
            we_all = cp.tile([P, NT], F32)       # per-token weight for this expert
            pos_all = cp.tile([P, NT], I32)      # compacted position (or BIG)
            xg_dram = dp.tile([C, REC], F32)     # gathered token records

            # ---------------- Phase A: router on all tokens (fp32) -------------
            with (
                nc.named_scope("A_router"),
                tc.tile_pool(name="pA", bufs=1) as pA,
                tc.tile_pool(name="xrow", bufs=3) as xrowp,
                tc.tile_pool(name="smallA", bufs=4) as smA,
                tc.tile_pool(name="psA", bufs=2, space="PSUM") as psA,
                tc.tile_pool(name="psR", bufs=2, space="PSUM") as psR,
            ):
                logits = pA.tile([P, NT, E], F32)
                sel_all = pA.tile([P, NT], F32)
                for tt in range(NT):
                    xrow = xrowp.tile([P, H], F32, tag="xrow")
                    nc.sync.dma_start(xrow[:], x_d[tt * P : (tt + 1) * P, :])
                    # transpose this token tile: xT_t[:, ht, :] = x[tt-tile].T
                    xT_t = xrowp.tile([P, NH, P], F32, tag="xTt")
                    for ht in range(NH):
                        pst = psA.tile([P, P], F32, tag="ptr")
                        nc.tensor.transpose(
                            pst[:], xrow[:, ht * P : (ht + 1) * P], ident[:]
                        )
                        nc.vector.tensor_copy(xT_t[:, ht, :], pst[:])
                    psl = psR.tile([P, E], F32, tag="plog")
                    for kt in range(NH):
                        nc.tensor.matmul(
                            psl[:],
                            lhsT=xT_t[:, kt, :],
                            rhs=gwt_sb[:, kt, :],
                            start=(kt == 0),
                            stop=(kt == NH - 1),
                        )
                    nc.vector.tensor_copy(logits[:, tt, :], psl[:])

                for tt in range(NT):
                    lg = logits[:, tt, :]
                    mx = smA.tile([P, 8], F32, tag="mx")
                    nc.vector.max(out=mx[:], in_=lg)
                    l1 = mx[:, 0:1]
                    l2 = mx[:, 1:2]
                    d12 = smA.tile([P, 1], F32, tag="d12")
                    nc.vector.tensor_sub(d12[:], l1, l2)
                    wa = smA.tile([P, 1], F32, tag="wa")
                    # wa = p1/(p1+p2) = sigmoid(l1 - l2); wb = 1 - wa
                    nc.scalar.activation(wa[:], d12[:], AF.Sigmoid)
                    wb = smA.tile([P, 1], F32, tag="wb")
                    nc.vector.tensor_scalar(
                        wb[:], wa[:], -1.0, 1.0, ALU.mult, ALU.add
                    )
                    # pe = logit of this core's expert (dot with one-hot)
                    tmp8 = smA.tile([P, E], F32, tag="tmp8")
                    nc.vector.tensor_tensor(tmp8[:], lg, eh_sb[:], ALU.mult)
                    pe = smA.tile([P, 1], F32, tag="pe")
                    nc.vector.reduce_sum(pe[:], tmp8[:], axis=mybir.AxisListType.X)
                    sel = smA.tile([P, 1], F32, tag="sel")
                    nc.vector.tensor_tensor(sel[:], pe[:], l2, ALU.is_ge)
                    is1 = smA.tile([P, 1], F32, tag="is1")
                    nc.vector.tensor_tensor(is1[:], pe[:], l1, ALU.is_ge)
                    # we = is1*wa + (sel-is1)*wb
                    t1 = smA.tile([P, 1], F32, tag="t1")
                    nc.vector.tensor_tensor(t1[:], is1[:], wa[:], ALU.mult)
                    t2 = smA.tile([P, 1], F32, tag="t2")
                    nc.vector.tensor_sub(t2[:], sel[:], is1[:])
                    nc.vector.tensor_tensor(t2[:], t2[:], wb[:], ALU.mult)
                    nc.vector.tensor_add(we_all[:, tt : tt + 1], t1[:], t2[:])
                    nc.vector.tensor_copy(sel_all[:, tt : tt + 1], sel[:])

                # cumulative positions across all 2048 tokens for this expert
                for tt in range(NT):
                    psc = psR.tile([P, 1], F32, tag="pcum")
                    for kt in range(tt):
                        nc.tensor.matmul(
                            psc[:],
                            lhsT=ones[:],
                            rhs=sel_all[:, kt : kt + 1],
                            start=(kt == 0),
                            stop=False,
                        )
                    nc.tensor.matmul(
                        psc[:],
                        lhsT=tri_sb[:],
                        rhs=sel_all[:, tt : tt + 1],
                        start=(tt == 0),
                        stop=True,
                    )
                    # posm = sel*(pos_incl - 1 - BIG) + BIG  (BIG when unselected)
                    pm = smA.tile([P, 1], F32, tag="pm")
                    nc.vector.tensor_scalar(
                        pm[:], psc[:], -1.0 - BIG, None, ALU.add
                    )
                    nc.vector.tensor_tensor(
                        pm[:], pm[:], sel_all[:, tt : tt + 1], ALU.mult
                    )
                    nc.vector.tensor_scalar(pm[:], pm[:], BIG, None, ALU.add)
                    nc.vector.tensor_copy(pos_all[:, tt : tt + 1], pm[:])

            # ------------- Phase A2: scatter token records to xg_dram ----------
            with (
                nc.named_scope("A2_scatter"),
                tc.tile_pool(name="recp", bufs=3) as recp,
                tc.tile_pool(name="smallB", bufs=2) as smB,
            ):
                patt = smB.tile([P, REC], F32, tag="patt")
                nc.vector.memset(patt[:], 0.0)
                nc.vector.memset(patt[:, 2049:2050], float(T))
                for g in range(NC_TILES):
                    nc.sync.dma_start(xg_dram[g * P : (g + 1) * P, :], patt[:])
                for tt in range(NT):
                    rec = recp.tile([P, REC], F32, tag="rec")
                    nc.sync.dma_start(
                        rec[:, 0:H], x_d[tt * P : (tt + 1) * P, :]
                    )
                    nc.vector.tensor_copy(
                        rec[:, 2048:2049], we_all[:, tt : tt + 1]
                    )
                    ti = smB.tile([P, 1], I32, tag="ti")
                    nc.gpsimd.iota(
                        ti[:], pattern=[[0, 1]], base=tt * P, channel_multiplier=1
                    )
                    nc.vector.tensor_copy(rec[:, 2049:2050], ti[:])
                    nc.gpsimd.indirect_dma_start(
                        out=xg_dram[:],
                        out_offset=IndirectOffsetOnAxis(
                            ap=pos_all[:, tt : tt + 1], axis=0
                        ),
                        in_=rec[:],
                        in_offset=None,
                        bounds_check=C - 1,
                        oob_is_err=False,
                    )

            # ---------------- Phase B: expert MLP on gathered tokens -----------
            wg_sb = cp.tile([P, NC_TILES], F32)
            idxg = cp.tile([P, NC_TILES], I32)
            with tc.tile_pool(name="mTp", bufs=1) as mTp:
                mT = mTp.tile([P, NI, C], BF16)  # silu(x@w1T)*(x@w3T), transp.
                with tc.tile_pool(name="xTgp", bufs=1) as xTgp:
                    xTg = xTgp.tile([P, NH, C], BF16)
                    with (
                        nc.named_scope("B_gather"),
                        tc.tile_pool(name="gat", bufs=2) as gat,
                        tc.tile_pool(name="psG", bufs=3, space="PSUM") as psG,
                    ):
                        for g in range(NC_TILES):
                            xgw = gat.tile([P, REC], F32, tag="xgw")
                            nc.sync.dma_start(
                                xgw[:], xg_dram[g * P : (g + 1) * P, :]
                            )
                            nc.vector.tensor_copy(
                                wg_sb[:, g : g + 1], xgw[:, 2048:2049]
                            )
                            nc.vector.tensor_copy(
                                idxg[:, g : g + 1], xgw[:, 2049:2050]
                            )
                            xbf = gat.tile([P, H], BF16, tag="xbf")
                            nc.vector.tensor_copy(xbf[:], xgw[:, 0:H])
                            for ht in range(NH):
                                pst = psG.tile([P, P], BF16, tag="ptrB")
                                nc.tensor.transpose(
                                    pst[:],
                                    xbf[:, ht * P : (ht + 1) * P],
                                    ident_bf[:],
                                )
                                nc.vector.tensor_copy(
                                    xTg[:, ht, g * P : (g + 1) * P], pst[:]
                                )

                    # ---- Stage 1: h = silu(x@w1T), g = x@w3T, mT = h*g -------
                    with (
                        nc.named_scope("B_stage1"),
                        tc.tile_pool(name="w13", bufs=2) as w13p,
                        tc.tile_pool(name="sil", bufs=3) as silp,
                        tc.tile_pool(name="ps1", bufs=2, space="PSUM") as ps1,
                    ):
                        for ib in range(NIB):
                            w1s = w13p.tile([P, NH, IB], BF16, tag="w1s")
                            nc.sync.dma_start(
                                w1s[:], w1p_d[ib].rearrange("k p i -> p k i")
                            )
                            w3s = w13p.tile([P, NH, IB], BF16, tag="w3s")
                            nc.sync.dma_start(
                                w3s[:], w3p_d[ib].rearrange("k p i -> p k i")
                            )
                            for it4 in range(IB // P):
                                it = ib * (IB // P) + it4
                                ph = ps1.tile([P, C], F32, tag="ph")
                                pg = ps1.tile([P, C], F32, tag="pg")
                                for kt in range(NH):
                                    lhs1 = w1s[:, kt, it4 * P : (it4 + 1) * P]
                                    nc.tensor.matmul(
                                        ph[:, 0:512],
                                        lhsT=lhs1,
                                        rhs=xTg[:, kt, 0:512],
                                        start=(kt == 0),
                                        stop=(kt == NH - 1),
                                    )
                                    nc.tensor.matmul(
                                        ph[:, 512:C],
                                        lhsT=lhs1,
                                        rhs=xTg[:, kt, 512:C],
                                        start=(kt == 0),
                                        stop=(kt == NH - 1),
                                    )
                                for kt in range(NH):
                                    lhs3 = w3s[:, kt, it4 * P : (it4 + 1) * P]
                                    nc.tensor.matmul(
                                        pg[:, 0:512],
                                        lhsT=lhs3,
                                        rhs=xTg[:, kt, 0:512],
                                        start=(kt == 0),
                                        stop=(kt == NH - 1),
                                    )
                                    nc.tensor.matmul(
                                        pg[:, 512:C],
                                        lhsT=lhs3,
                                        rhs=xTg[:, kt, 512:C],
                                        start=(kt == 0),
                                        stop=(kt == NH - 1),
                                    )
                                hs = silp.tile([P, C], F32, tag="hs")
                                nc.scalar.activation(hs[:], ph[:], AF.Silu)
                                nc.vector.tensor_tensor(
                                    mT[:, it, :], hs[:], pg[:], ALU.mult
                                )

                # ---- Stage 2: out2 = (h*g) @ w2T, weight, scatter ------------
                with (
                    nc.named_scope("B_stage2"),
                    tc.tile_pool(name="w2", bufs=2) as w2p,
                    tc.tile_pool(name="o2", bufs=1) as o2p,
                    tc.tile_pool(name="ps2", bufs=4, space="PSUM") as ps2,
                ):
                    out2 = o2p.tile([P, NC_TILES, H], F32)
                    for jb in range(NJB):
                        w2s = w2p.tile([P, JB, H], BF16, tag="w2s")
                        nc.sync.dma_start(
                            w2s[:],
                            w2t_d[jb * JB * P : (jb + 1) * JB * P, :].rearrange(
                                "(b p) h -> p b h", p=P
                            ),
                        )
                        for mt in range(NC_TILES):
                            for nb in range(H // 512):
                                po = ps2.tile([P, 512], F32, tag="po")
                                for j in range(JB):
                                    nc.tensor.matmul(
                                        po[:],
                                        lhsT=mT[
                                            :, jb * JB + j, mt * P : (mt + 1) * P
                                        ],
                                        rhs=w2s[:, j, nb * 512 : (nb + 1) * 512],
                                        start=(j == 0),
                                        stop=(j == JB - 1),
                                    )
                                dst = out2[:, mt, nb * 512 : (nb + 1) * 512]
                                if jb == 0:
                                    nc.vector.tensor_copy(dst, po[:])
                                else:
                                    nc.vector.tensor_add(dst, dst, po[:])

                    for mt in range(NC_TILES):
                        nc.vector.tensor_tensor(
                            out2[:, mt, :],
                            out2[:, mt, :],
                            wg_sb[:, mt : mt + 1].to_broadcast([P, H]),
                            ALU.mult,
                        )
                        nc.gpsimd.indirect_dma_start(
                            out=out_d[:],
                            out_offset=IndirectOffsetOnAxis(
                                ap=idxg[:, mt : mt + 1], axis=0
                            ),
                            in_=out2[:, mt, :],
                            in_offset=None,
                            bounds_check=T - 1,
                            oob_is_err=False,
                        )

    nc.finalize()
    return nc


_NC_CACHE = None


def _get_nc():
    global _NC_CACHE
    if _NC_CACHE is None:
        _NC_CACHE = _build()
    return _NC_CACHE


def _prep_in_maps(hidden_states, gate_w, w1, w2, w3):
    x = np.ascontiguousarray(
        np.asarray(hidden_states, dtype=np.float32).reshape(T, H)
    )
    gate_w = np.asarray(gate_w, dtype=np.float32)
    # gwt[p, o, e] = gate_w[e, o*128+p]
    gwt = np.ascontiguousarray(
        gate_w.T.reshape(NH, P, E).transpose(1, 0, 2)
    )
    tri = np.triu(np.ones((P, P), np.float32))
    in_maps = []
    for e in range(N_CORES):
        eh = np.zeros((P, E), np.float32)
        eh[:, e] = 1.0
        w1t = np.asarray(w1[e], dtype=np.float32).astype(BF16_NP).T  # [H, I]
        w3t = np.asarray(w3[e], dtype=np.float32).astype(BF16_NP).T
        # pack [NIB, NH, P, IB]: w1p[ib, kt, p, i] = w1t[kt*P+p, ib*IB+i]
        w1p = np.ascontiguousarray(
            w1t.reshape(NH, P, NIB, IB).transpose(2, 0, 1, 3)
        )
        w3p = np.ascontiguousarray(
            w3t.reshape(NH, P, NIB, IB).transpose(2, 0, 1, 3)
        )
        w2t = np.ascontiguousarray(
            np.asarray(w2[e], dtype=np.float32).astype(BF16_NP).T
        )  # [I, H]
        in_maps.append(
            {
                "x": x,
                "gwt": gwt,
                "eh": eh,
                "tri": tri,
                "w1p": w1p,
                "w3p": w3p,
                "w2t": w2t,
            }
        )
    return in_maps


def kernel(hidden_states, gate_w, w1, w2, w3):
    nc = _get_nc()
    in_maps = _prep_in_maps(hidden_states, gate_w, w1, w2, w3)
    res = run_bass_kernel_spmd(nc, in_maps, core_ids=list(range(N_CORES)))
    out = np.zeros((T, H), np.float32)
    for r in res.results:
        out += r["out"]
    return out.reshape(np.asarray(hidden_states).shape).astype(np.float32)
